# revision 8
# baseline (speedup 1.0000x reference)
"""Trainium2 Bass kernel for a GQA causal-attention block (TP over heads, 8 cores).

Computation (per reference): q/k/v projections of x, interleaved RoPE on q/k,
GQA causal attention (32 q heads, 8 kv heads, head_dim 128, seq 2048), output
projection. Sharding: tensor-parallel over heads — each core owns 4 q heads and
their shared kv head. The attention output (transposed layout) is AllGathered
across cores per 512-sequence window and each core computes a 512-column slice
of the final output projection; the host concatenates the column slices.

Device dataflow notes:
  - Everything transposed: x^T streams as the matmul moving operand so q^T/k^T
    come out with head_dim on partitions; scores are computed transposed
    (s^T[k_pos, q_pos]) so exp(s^T) feeds the PV matmul directly as the moving
    operand without any on-chip transposes.
  - RoPE uses a half-split head_dim permutation (evens then odds), folded into
    the wq/wk columns on the host. The trig tables are duplicated per half on
    the host so the rotation is two full-width [128,W] multiplies (DVE) plus a
    cross-half sub/add pair. QKV matmuls run chain-major (k first, then q heads
    in attention order) so each rope fires as soon as its chain completes and
    attention never waits on the DVE.
  - Softmax skips the max subtraction (scores ~ N(0,1) after scaling). Row sums
    come from ones-matmuls over quad-accumulated exp tiles (two DVE add levels
    fold 4 kv tiles into one PE sum matmul). The ones stationary is [128,128],
    so the sum lands replicated on all 128 PSUM partitions — the reciprocal is
    taken full-width on the DVE and feeds the normalizing multiply directly (no
    PE broadcast needed).
  - Causal masking multiplies exp(scores) by a 0/1 mask on diagonal blocks;
    diagonal score/exp/PV work only covers the live trapezoid [c*128, W). The
    exp-tile buffers are memset once at startup so the first mask multiply
    never sees uninitialized SBUF (later reuses hold finite stale exps).
  - Windows run in natural order 0..3 so the per-head AllGathers of windows
    2/3 spread across the whole back half of the kernel instead of bunching in
    the tail; window 0/1 use one full-window AllGather each. Output projection
    for per-head-gathered windows contracts in head-arrival order.
  - Global software pipeline: QKV(w) prefetches x^T(w+1); the initial DMAs use
    a small first bite (2 contraction chunks) on xt/wq/wk so the first matmul
    issues ~7us earlier; window-0 loads are spread across three DMA queues to
    keep the chain-major k chain fed.
  - PSUM is exactly 8 banks: QKV uses 5 transient banks (k + 4 q chains),
    sharing with the 2 score banks, 1+1 PV/sum accumulators, and 2 v/wo banks.
"""

import numpy as np
import ml_dtypes

import concourse.bass as bass
import concourse.mybir as mybir
import concourse.tile as tile
from concourse import bacc
from concourse.bass_utils import run_bass_kernel_spmd

N_CORES = 8
P = 128
SEQ = 2048
DIM = 4096
N_HEADS = 32
N_KV_HEADS = 8
HD = 128
QH = N_HEADS // N_CORES        # q heads per core
KD = DIM // P                  # contraction chunks
KG = 4                         # k-chunk DMA groups
KPG = KD // KG                 # k chunks per group
W = 512                        # seq window (matmul moving free dim)
NW = SEQ // W
NT = SEQ // P
OUTC = DIM // N_CORES          # output columns per core
SCALE = HD ** -0.5

BF16 = mybir.dt.bfloat16
F32 = mybir.dt.float32

HEAD_ORDER = (2, 3, 0, 1)


def _build_nc():
    nc = bacc.Bacc("TRN2", target_bir_lowering=False, debug=False,
                   num_devices=N_CORES)

    xt_d = nc.dram_tensor("xt", [NW, P, KD, W], BF16, kind="ExternalInput")
    wq_d = nc.dram_tensor("wq", [P, KD, QH * HD], BF16, kind="ExternalInput")
    wk_d = nc.dram_tensor("wk", [P, KD, HD], BF16, kind="ExternalInput")
    wv_d = nc.dram_tensor("wv", [P, KD, HD], BF16, kind="ExternalInput")
    wo_d = nc.dram_tensor("wo", [P, KD, OUTC], BF16, kind="ExternalInput")
    cs_d = nc.dram_tensor("cs", [P, SEQ], BF16, kind="ExternalInput")
    sn_d = nc.dram_tensor("sn", [P, SEQ], BF16, kind="ExternalInput")
    out_d = nc.dram_tensor("out", [SEQ, OUTC], F32, kind="ExternalOutput")

    # 0/1 causal masks for the 4 diagonal alignments of a [128 kv, 512 q] block:
    # mask[p, c, q] = 1 iff kv offset p + c*128 <= q (within the 512-q window).
    j = np.arange(P)[:, None, None]
    c = np.arange(4)[None, :, None]
    q = np.arange(W)[None, None, :]
    masks_np = (j + c * P <= q).astype(ml_dtypes.bfloat16)
    masks_d = nc.inline_tensor(masks_np, "cmasks")
    ones_d = nc.inline_tensor(np.ones([P, P], dtype=ml_dtypes.bfloat16), "ones")

    Exp = mybir.ActivationFunctionType.Exp

    from contextlib import ExitStack
    with tile.TileContext(nc) as tc:
        with tc.tile_pool(name="const", bufs=1) as constp, \
             tc.tile_pool(name="acts", bufs=1) as actp, \
             tc.tile_pool(name="ropet", bufs=1) as rtp, \
             tc.tile_pool(name="ccdram", bufs=1, space="DRAM") as dcc, \
             tc.tile_pool(name="wop", bufs=1) as wop:
            region_a = ExitStack()
            wp = region_a.enter_context(tc.tile_pool(name="wqkv", bufs=1, side="right"))
            xp = region_a.enter_context(tc.tile_pool(name="xtp", bufs=2, side="right"))
            qrot = actp.tile([P, QH, SEQ], BF16)   # q^T (rope'd), per head
            krot = actp.tile([P, SEQ], BF16)       # k^T (rope'd)
            v_sb = actp.tile([P, NT, HD], BF16)    # v row-tiles [seq, hd]

            wq_g = [wp.tile([P, KPG, QH * HD], BF16, name=f"wq{g}")
                    for g in range(KG)]
            wk_g = [wp.tile([P, KPG, HD], BF16, name=f"wk{g}")
                    for g in range(KG)]
            wv_g = [wp.tile([P, KPG, HD], BF16, name=f"wv{g}")
                    for g in range(KG)]
            xt_gs: dict[tuple[int, int], bass.AP] = {}

            def load_xt_group(w, g, eng=None):
                t = xp.tile([P, KPG, W], BF16, tag=f"xt{g}")
                if eng is None:
                    eng = nc.sync if g % 2 == 0 else nc.scalar
                eng.dma_start(
                    out=t[:], in_=xt_d[w, :, g * KPG:(g + 1) * KPG, :])
                xt_gs[(w, g)] = t

            def grp(handle, g):
                return handle[:, g * KPG:(g + 1) * KPG, :]

            # Kick the collective-stream init (a ~60us one-time barrier on
            # the CC cores) immediately: a 4-byte AllGather of never-written
            # scratch emits the barrier before the DMA setups occupy the
            # gpsimd queue, so it no longer gates the first real AllGather.
            ccw_in = dcc.tile([1, 4], BF16, name="ccwarm_i")
            ccw_out = dcc.tile([N_CORES, 4], BF16, addr_space="Shared",
                               name="ccwarm_o")
            nc.gpsimd.collective_compute(
                "AllGather", mybir.AluOpType.bypass,
                replica_groups=[list(range(N_CORES))],
                ins=[ccw_in[:]], outs=[ccw_out[:]])

            # Initial DMAs. The window-0 supply is split: a small first bite
            # on xt/wq/wk lets matmul 0 issue as soon as possible, and group 3
            # of x^T rides the (otherwise idle) gpsimd queue so consumption
            # never outruns the sync queue.
            t0x = xp.tile([P, KPG, W], BF16, tag="xt0")
            nc.sync.dma_start(out=t0x[:, 0:2, :], in_=xt_d[0, :, 0:2, :])
            nc.sync.dma_start(out=t0x[:, 2:KPG, :], in_=xt_d[0, :, 2:KPG, :])
            xt_gs[(0, 0)] = t0x
            nc.gpsimd.dma_start(out=wk_g[0][:, 0:2, :], in_=wk_d[:, 0:2, :])
            nc.gpsimd.dma_start(out=wk_g[0][:, 2:KPG, :], in_=wk_d[:, 2:KPG, :])
            nc.scalar.dma_start(out=wq_g[0][:, 0:2, :], in_=wq_d[:, 0:2, :])
            nc.scalar.dma_start(out=wq_g[0][:, 2:KPG, :], in_=wq_d[:, 2:KPG, :])
            for g in (1, 2):
                load_xt_group(0, g, eng=nc.sync)
            for g in range(1, KG):
                nc.gpsimd.dma_start(out=wk_g[g][:], in_=grp(wk_d, g))
                nc.scalar.dma_start(out=wq_g[g][:], in_=grp(wq_d, g))
            load_xt_group(0, 3, eng=nc.gpsimd)
            cc_sb = constp.tile([P, SEQ], BF16)
            ss_sb = constp.tile([P, SEQ], BF16)
            nc.gpsimd.dma_start(out=cc_sb[:], in_=cs_d[:])
            nc.gpsimd.dma_start(out=ss_sb[:], in_=sn_d[:])
            for g in range(KG):
                nc.gpsimd.dma_start(out=wv_g[g][:], in_=grp(wv_d, g))
            load_xt_group(1, 0, eng=nc.sync)
            load_xt_group(1, 2, eng=nc.sync)
            load_xt_group(1, 1, eng=nc.scalar)
            load_xt_group(1, 3, eng=nc.scalar)
            masks_sb = constp.tile([P, 4, W], BF16)
            nc.scalar.dma_start(out=masks_sb[:], in_=masks_d[:])
            ones_sb = constp.tile([P, P], BF16)
            nc.scalar.dma_start(out=ones_sb[:], in_=ones_d[:])
            wo_sb = wop.tile([P, KD, OUTC], BF16)

            ag_in = [dcc.tile([QH * HD, W], BF16, name=f"agin{w}")
                     for w in range(NW)]
            ag_out = {w: dcc.tile([DIM, W], BF16, addr_space="Shared",
                                  name=f"agout{w}")
                      for w in (0, 1)}
            # windows 2/3 AllGather per-head: small collectives that drain on
            # the serial CC stream while attention still computes
            PERHEAD_AG = (2, 3)
            ag_outh = {w: [dcc.tile([N_CORES * HD, W], BF16,
                                    addr_space="Shared",
                                    name=f"agout{w}h{h}")
                           for h in range(QH)]
                       for w in PERHEAD_AG}

            def rope(ps, dst, s0):
                # ps rows 0:64 = even dims (x1), 64:128 = odd dims (x2);
                # cc/ss hold [cos;cos] and [sin;sin]. The Activation engine
                # evicts the psum twice — straight (psb) and half-swapped
                # (psw = [x2; x1], PSUM->SBUF copies may shift the base
                # partition) — so the DVE does the whole rotation as two
                # full-width bf16 multiplies (2x DVE rate) plus the halved
                # sub/add, all on equal-base SBUF operands.
                cw = cc_sb[:, s0:s0 + W]
                sw = ss_sb[:, s0:s0 + W]
                psb = rtp.tile([P, W], BF16, tag="psb")
                psw = rtp.tile([P, W], BF16, tag="psw")
                nc.scalar.copy(psb[:], ps[:])
                nc.scalar.copy(psw[0:64, :], ps[64:128, :])
                nc.scalar.copy(psw[64:128, :], ps[0:64, :])
                t1 = rtp.tile([P, W], BF16, tag="t1")
                t2 = rtp.tile([P, W], BF16, tag="t2")
                nc.vector.tensor_mul(t1[:], psb[:], cw)      # [x1*c ; x2*c]
                nc.vector.tensor_mul(t2[:], psw[:], sw)      # [x2*s ; x1*s]
                nc.vector.tensor_sub(dst[0:64], t1[0:64, :], t2[0:64, :])
                nc.vector.tensor_add(dst[64:128], t1[64:128, :], t2[64:128, :])

            # steady-state psum pools: exactly 8 banks, with QKV/scores and
            # v/wo sharing slots (the streams interleave on the PE anyway).
            with tc.tile_pool(name="pmixs", bufs=2, space="PSUM") as pmixs, \
                 tc.tile_pool(name="pmixv", bufs=2, space="PSUM") as pmixv, \
                 tc.tile_pool(name="pso", bufs=2, space="PSUM") as pso, \
                 tc.tile_pool(name="pssum", bufs=2, space="PSUM") as pss, \
                 tc.tile_pool(name="ptp", bufs=4) as ptp, \
                 tc.tile_pool(name="sumt", bufs=3) as sumt, \
                 tc.tile_pool(name="attp", bufs=2) as attp:

                # zero the exp-tile buffers so the first diagonal mask
                # multiply never reads uninitialized SBUF (see docstring)
                for _ in range(4):
                    zt = ptp.tile([P, W], BF16, tag="pt")
                    nc.vector.memset(zt[:], 0.0)

                def emit_v(w):
                    for mm in range(W // P):
                        psv = pmixv.tile([P, OUTC], F32, tag="vw")
                        for k in range(KD):
                            g, kk = k // KPG, k % KPG
                            nc.tensor.matmul(
                                psv[:, 0:HD],
                                xt_gs[(w, g)][:, kk, mm * P:(mm + 1) * P],
                                wv_g[g][:, kk, :],
                                start=(k == 0), stop=(k == KD - 1))
                        nc.scalar.copy(v_sb[:, w * (W // P) + mm, :],
                                       psv[:, 0:HD])

                def emit_qkv_gouter(w):
                    # Window 0 runs contraction-major (k,q2,q3,q0,q1 per
                    # chunk) so PE consumption paces the initial DMA supply;
                    # later windows run chain-major (k chain first, then q
                    # heads in attention order) so each rope fires as soon as
                    # its chain completes. v last in both.
                    s0 = w * W
                    if w == 0:
                        pass  # window-1 groups already issued in the preamble
                    elif w + 1 < NW:
                        for g in range(KG):
                            load_xt_group(w + 1, g)
                    ps_k = pmixs.tile([P, W], F32, tag="s", name="kproj")
                    ps_q = {2: pso.tile([P, W], F32, tag="o", name="q2"),
                            3: pso.tile([P, W], F32, tag="o", name="q3"),
                            0: pss.tile([P, W], F32, tag="sum", name="q0"),
                            1: pss.tile([P, W], F32, tag="sum", name="q1")}

                    def kmm(g, kk):
                        k = g * KPG + kk
                        nc.tensor.matmul(ps_k[:], wk_g[g][:, kk, :],
                                         xt_gs[(w, g)][:, kk, :],
                                         start=(k == 0), stop=(k == KD - 1))

                    def qmm(m, g, kk):
                        k = g * KPG + kk
                        nc.tensor.matmul(ps_q[m][:],
                                         wq_g[g][:, kk, m * HD:(m + 1) * HD],
                                         xt_gs[(w, g)][:, kk, :],
                                         start=(k == 0), stop=(k == KD - 1))

                    if w == 0:
                        for g in range(KG):
                            for kk in range(KPG):
                                kmm(g, kk)
                                for m in HEAD_ORDER:
                                    qmm(m, g, kk)
                        rope(ps_k, krot[:, s0:s0 + W], s0)
                        for m in HEAD_ORDER:
                            rope(ps_q[m], qrot[:, m, s0:s0 + W], s0)
                    else:
                        for g in range(KG):
                            for kk in range(KPG):
                                kmm(g, kk)
                        rope(ps_k, krot[:, s0:s0 + W], s0)
                        for m in HEAD_ORDER:
                            for g in range(KG):
                                for kk in range(KPG):
                                    qmm(m, g, kk)
                            rope(ps_q[m], qrot[:, m, s0:s0 + W], s0)
                    emit_v(w)

                def emit_attention(qc):
                    q0 = qc * W
                    nkv = (W // P) * (qc + 1)
                    nquads = nkv // 4

                    for h in HEAD_ORDER:
                        ps_o = pso.tile([P, W], F32, tag="o")
                        ps_sum = pss.tile([P, W], F32, tag="sum")
                        for qd in range(nquads):
                            pts = []
                            pa = None
                            for ji in range(4):
                                jj = 4 * qd + ji
                                j0 = jj * P
                                cdiag = jj - (W // P) * qc
                                # live trapezoid of a diagonal block
                                lo = cdiag * P if cdiag > 0 else 0
                                ps_s = pmixs.tile([P, W], F32, tag="s")
                                nc.tensor.matmul(ps_s[:, lo:W],
                                                 krot[:, j0:j0 + P],
                                                 qrot[:, h, q0 + lo:q0 + W],
                                                 start=True, stop=True)
                                pt = ptp.tile([P, W], BF16, tag="pt")
                                nc.scalar.activation(pt[:, lo:W], ps_s[:, lo:W],
                                                     Exp, scale=SCALE)
                                if cdiag >= 0:
                                    # zeroes the masked triangle AND any stale
                                    # [0:lo) prefix left from buffer reuse
                                    nc.vector.tensor_mul(
                                        pt[:], pt[:], masks_sb[:, cdiag, :])
                                nc.tensor.matmul(ps_o[:, lo:W],
                                                 v_sb[:, jj, :], pt[:, lo:W],
                                                 start=(jj == 0),
                                                 stop=(jj == nkv - 1))
                                pts.append(pt)
                                if ji == 1:
                                    # pa/pb ride the (mostly idle) gpsimd
                                    # engine so the DVE keeps pace with the
                                    # PE during attention windows
                                    pa = sumt.tile([P, W], BF16, tag="pta")
                                    nc.gpsimd.tensor_add(pa[:], pts[0][:],
                                                         pts[1][:])
                            pb = sumt.tile([P, W], BF16, tag="pta")
                            nc.gpsimd.tensor_add(pb[:], pts[2][:], pts[3][:])
                            pq = sumt.tile([P, W], BF16, tag="pta")
                            nc.vector.tensor_add(pq[:], pa[:], pb[:])
                            nc.tensor.matmul(ps_sum[:], ones_sb[:], pq[:],
                                             start=(qd == 0),
                                             stop=(qd == nquads - 1))
                        # the ones stationary replicates the row sum across
                        # all 128 partitions — reciprocal full-width, no
                        # PE broadcast needed
                        inv_bc = attp.tile([P, W], F32, tag="invbc")
                        nc.vector.reciprocal_approx_fast(out=inv_bc[:],
                                                         in_=ps_sum[:])
                        at = attp.tile([P, W], BF16, tag="at", bufs=3)
                        nc.vector.tensor_mul(at[:], ps_o[:], inv_bc[:])
                        nc.sync.dma_start(out=ag_in[qc][h * HD:(h + 1) * HD, :],
                                          in_=at[:])
                        if qc in PERHEAD_AG:
                            nc.gpsimd.collective_compute(
                                "AllGather", mybir.AluOpType.bypass,
                                replica_groups=[list(range(N_CORES))],
                                ins=[ag_in[qc][h * HD:(h + 1) * HD, :]],
                                outs=[ag_outh[qc][h][:]])
                    if qc not in PERHEAD_AG:
                        nc.gpsimd.collective_compute(
                            "AllGather", mybir.AluOpType.bypass,
                            replica_groups=[list(range(N_CORES))],
                            ins=[ag_in[qc][:]], outs=[ag_out[qc][:]])

                atfs: dict[int, bass.AP] = {}

                def load_atf(qc):
                    # atf DMAs ride the gpsimd queue, emitted after every
                    # AllGather trigger: their AG-completion waits then only
                    # head-of-line-block each other, never the exp stream
                    # (scalar) or the collective doorbells
                    atf = atfp.tile([P, KD, W], BF16, tag="atf")
                    if qc in PERHEAD_AG:
                        # chunk k of atf = global dim block; per-head gathers
                        # hold (core, head h) blocks, i.e. chunks k ≡ h (mod QH)
                        atf4 = atf.rearrange("p (c h) n -> p c h n", h=QH)
                        for h in HEAD_ORDER:   # ag completion order
                            nc.gpsimd.dma_start(
                                out=atf4[:, :, h, :],
                                in_=ag_outh[qc][h][:].rearrange(
                                    "(c p) n -> p c n", p=P))
                    else:
                        nc.gpsimd.dma_start(
                            out=atf[:],
                            in_=ag_out[qc][:].rearrange("(k p) n -> p k n", p=P))
                    atfs[qc] = atf

                def emit_wo(qc):
                    q0 = qc * W
                    atf = atfs.pop(qc)
                    # per-head-gathered windows: contract in head-arrival order
                    # so each chain starts as soon as the first plane lands
                    # (summation order is irrelevant to the result)
                    korder = ([k for h in HEAD_ORDER
                               for k in range(h, KD, QH)]
                              if qc in PERHEAD_AG else list(range(KD)))
                    for mm in range(W // P):
                        m0 = mm * P
                        ps = pmixv.tile([P, OUTC], F32, tag="vw")
                        for j, k in enumerate(korder):
                            nc.tensor.matmul(ps[:],
                                             atf[:, k, m0:m0 + P],
                                             wo_sb[:, k, :],
                                             start=(j == 0), stop=(j == KD - 1))
                        ot = outp.tile([P, OUTC], F32, tag="ot")
                        nc.vector.tensor_copy(ot[:], ps[:])
                        nc.scalar.dma_start(out=out_d[q0 + m0:q0 + m0 + P, :],
                                            in_=ot[:])

                emit_qkv_gouter(0)
                emit_attention(0)
                emit_qkv_gouter(1)
                emit_attention(1)
                emit_qkv_gouter(2)
                emit_attention(2)
                nc.scalar.dma_start(out=wo_sb[:], in_=wo_d[:])
                emit_qkv_gouter(3)
                region_a.close()   # frees the weight/x^T pools for atf staging
                region_b = ExitStack()
                atfp = region_b.enter_context(tc.tile_pool(name="atfp", bufs=3, side="right"))
                outp = region_b.enter_context(tc.tile_pool(name="outp", bufs=3, side="right"))
                emit_attention(3)
                load_atf(0)
                load_atf(1)
                load_atf(2)
                load_atf(3)
                emit_wo(0)
                emit_wo(1)
                emit_wo(2)
                emit_wo(3)
                region_b.close()

    nc.compile()
    return nc


_NC_CACHE = None


def _get_nc():
    global _NC_CACHE
    if _NC_CACHE is None:
        _NC_CACHE = _build_nc()
    return _NC_CACHE


def make_in_maps(x, freqs_cos, freqs_sin, wq, wk, wv, wo):
    bf16 = ml_dtypes.bfloat16
    # half-split permutation: evens then odds within each head's 128 dims
    pidx = np.concatenate([np.arange(0, HD, 2), np.arange(1, HD, 2)])

    def ptile(w):
        # [DIM, cols] -> [P, KD, cols] so per-partition DMA runs are contiguous
        return np.ascontiguousarray(
            w.reshape(KD, P, w.shape[1]).transpose(1, 0, 2).astype(bf16))

    xt = np.ascontiguousarray(
        np.asarray(x).T.astype(bf16).reshape(KD, P, NW, W).transpose(2, 1, 0, 3))
    cs1 = np.asarray(freqs_cos).T.astype(np.float32)   # [64, SEQ]
    sn1 = np.asarray(freqs_sin).T.astype(np.float32)
    cs = np.ascontiguousarray(np.concatenate([cs1, cs1], axis=0).astype(bf16))
    sn = np.ascontiguousarray(np.concatenate([sn1, sn1], axis=0).astype(bf16))
    wq = np.asarray(wq)
    wk = np.asarray(wk)
    wv = np.asarray(wv)
    wo = np.asarray(wo)
    in_maps = []
    for core in range(N_CORES):
        q_cols = np.concatenate([h * HD + pidx
                                 for h in range(QH * core, QH * (core + 1))])
        in_maps.append({
            "xt": xt,
            "wq": ptile(wq[:, q_cols]),
            "wk": ptile(wk[:, core * HD + pidx]),
            "wv": ptile(wv[:, core * HD:(core + 1) * HD]),
            "wo": ptile(wo[:, core * OUTC:(core + 1) * OUTC]),
            "cs": cs,
            "sn": sn,
        })
    return in_maps


def kernel(x, freqs_cos, freqs_sin, wq, wk, wv, wo, _run_kwargs=None):
    in_maps = make_in_maps(x, freqs_cos, freqs_sin, wq, wk, wv, wo)
    nc = _get_nc()
    res = run_bass_kernel_spmd(nc, in_maps, list(range(N_CORES)),
                               **(_run_kwargs or {}))
    out = np.concatenate([res.results[i]["out"] for i in range(N_CORES)], axis=1)
    if _run_kwargs is not None:
        kernel.last_results = res
    return np.ascontiguousarray(out.astype(np.float32))


# revision 14
# speedup vs baseline: 1.1173x; 1.1173x over previous
"""Trainium2 Bass kernel for a GQA causal-attention block (TP over heads, 8 cores).

Computation (per reference): q/k/v projections of x, interleaved RoPE on q/k,
GQA causal attention (32 q heads, 8 kv heads, head_dim 128, seq 2048), output
projection. Sharding: tensor-parallel over heads — each core owns 4 q heads and
their shared kv head. The attention output (transposed layout) is AllGathered
across cores per 512-sequence window and each core computes a 512-column slice
of the final output projection; the host concatenates the column slices.

Device dataflow notes:
  - Everything transposed: x^T streams as the matmul moving operand so q^T/k^T
    come out with head_dim on partitions; scores are computed transposed
    (s^T[k_pos, q_pos]) so exp(s^T) feeds the PV matmul directly as the moving
    operand without any on-chip transposes.
  - RoPE uses a half-split head_dim permutation (evens then odds), folded into
    the wq/wk columns on the host. The trig tables are duplicated per half on
    the host so the rotation is two full-width [128,W] multiplies (DVE) plus a
    cross-half sub/add pair. QKV matmuls run chain-major (k first, then q heads
    in attention order) so each rope fires as soon as its chain completes and
    attention never waits on the DVE.
  - Softmax skips the max subtraction (scores ~ N(0,1) after scaling). Row sums
    come from ones-matmuls over quad-accumulated exp tiles (two DVE add levels
    fold 4 kv tiles into one PE sum matmul). The ones stationary is [128,128],
    so the sum lands replicated on all 128 PSUM partitions — the reciprocal is
    taken full-width on the DVE and feeds the normalizing multiply directly (no
    PE broadcast needed).
  - Causal masking multiplies exp(scores) by a 0/1 mask on diagonal blocks;
    diagonal score/exp/PV work only covers the live trapezoid [c*128, W). The
    exp-tile buffers are memset once at startup so the first mask multiply
    never sees uninitialized SBUF (later reuses hold finite stale exps).
  - Windows run in natural order 0..3 so the per-head AllGathers of windows
    2/3 spread across the whole back half of the kernel instead of bunching in
    the tail; window 0/1 use one full-window AllGather each. Output projection
    for per-head-gathered windows contracts in head-arrival order.
  - Global software pipeline: QKV(w) prefetches x^T(w+1); the initial DMAs use
    a small first bite (2 contraction chunks) on xt/wq/wk so the first matmul
    issues ~7us earlier; window-0 loads are spread across three DMA queues to
    keep the chain-major k chain fed.
  - PSUM is exactly 8 banks: QKV uses 5 transient banks (k + 4 q chains),
    sharing with the 2 score banks, 1+1 PV/sum accumulators, and 2 v/wo banks.
"""

import numpy as np
import ml_dtypes

import concourse.bass as bass
import concourse.mybir as mybir
import concourse.tile as tile
from concourse import bacc
from concourse.bass_utils import run_bass_kernel_spmd

N_CORES = 8
P = 128
SEQ = 2048
DIM = 4096
N_HEADS = 32
N_KV_HEADS = 8
HD = 128
QH = N_HEADS // N_CORES        # q heads per core
KD = DIM // P                  # contraction chunks
KG = 4                         # k-chunk DMA groups
KPG = KD // KG                 # k chunks per group
W = 512                        # seq window (matmul moving free dim)
NW = SEQ // W
NT = SEQ // P
OUTC = DIM // N_CORES          # output columns per core
SCALE = HD ** -0.5

BF16 = mybir.dt.bfloat16
F32 = mybir.dt.float32

HEAD_ORDER = (2, 3, 0, 1)


def _build_nc():
    nc = bacc.Bacc("TRN2", target_bir_lowering=False, debug=False,
                   num_devices=N_CORES)

    xt_d = nc.dram_tensor("xt", [NW, P, KD, W], BF16, kind="ExternalInput")
    wq_d = nc.dram_tensor("wq", [P, KD, QH * HD], BF16, kind="ExternalInput")
    wk_d = nc.dram_tensor("wk", [P, KD, HD], BF16, kind="ExternalInput")
    wv_d = nc.dram_tensor("wv", [P, KD, HD], BF16, kind="ExternalInput")
    wo_d = nc.dram_tensor("wo", [P, KD, OUTC], BF16, kind="ExternalInput")
    cs_d = nc.dram_tensor("cs", [P, SEQ], BF16, kind="ExternalInput")
    sn_d = nc.dram_tensor("sn", [P, SEQ], BF16, kind="ExternalInput")
    out_d = nc.dram_tensor("out", [SEQ, OUTC], F32, kind="ExternalOutput")

    # 0/1 causal masks for the 4 diagonal alignments of a [128 kv, 512 q] block:
    # mask[p, c, q] = 1 iff kv offset p + c*128 <= q (within the 512-q window).
    j = np.arange(P)[:, None, None]
    c = np.arange(4)[None, :, None]
    q = np.arange(W)[None, None, :]
    masks_np = (j + c * P <= q).astype(ml_dtypes.bfloat16)
    masks_d = nc.inline_tensor(masks_np, "cmasks")
    ones_d = nc.inline_tensor(np.ones([P, P], dtype=ml_dtypes.bfloat16), "ones")

    Exp = mybir.ActivationFunctionType.Exp

    from contextlib import ExitStack
    with tile.TileContext(nc) as tc:
        with tc.tile_pool(name="const", bufs=1) as constp, \
             tc.tile_pool(name="acts", bufs=1) as actp, \
             tc.tile_pool(name="ropet", bufs=1) as rtp, \
             tc.tile_pool(name="ccdram", bufs=1, space="DRAM") as dcc, \
             tc.tile_pool(name="wop", bufs=1) as wop:
            region_a = ExitStack()
            wp = region_a.enter_context(tc.tile_pool(name="wqkv", bufs=1, side="right"))
            xp = region_a.enter_context(tc.tile_pool(name="xtp", bufs=2, side="right"))
            qrot = actp.tile([P, QH, SEQ], BF16)   # q^T (rope'd), per head
            krot = actp.tile([P, SEQ], BF16)       # k^T (rope'd)
            v_sb = actp.tile([P, NT, HD], BF16)    # v row-tiles [seq, hd]

            wq_g = [wp.tile([P, KPG, QH * HD], BF16, name=f"wq{g}")
                    for g in range(KG)]
            wk_g = [wp.tile([P, KPG, HD], BF16, name=f"wk{g}")
                    for g in range(KG)]
            wv_g = [wp.tile([P, KPG, HD], BF16, name=f"wv{g}")
                    for g in range(KG)]
            xt_gs: dict[tuple[int, int], bass.AP] = {}

            def load_xt_group(w, g, eng=None):
                t = xp.tile([P, KPG, W], BF16, tag=f"xt{g}")
                if eng is None:
                    eng = nc.sync if g % 2 == 0 else nc.scalar
                eng.dma_start(
                    out=t[:], in_=xt_d[w, :, g * KPG:(g + 1) * KPG, :])
                xt_gs[(w, g)] = t

            def grp(handle, g):
                return handle[:, g * KPG:(g + 1) * KPG, :]

            # Initial DMAs. The window-0 supply is split: a small first bite
            # on xt/wq/wk lets matmul 0 issue as soon as possible, and group 3
            # of x^T rides the (otherwise idle) gpsimd queue so consumption
            # never outruns the sync queue.
            t0x = xp.tile([P, KPG, W], BF16, tag="xt0")
            nc.sync.dma_start(out=t0x[:, 0:2, :], in_=xt_d[0, :, 0:2, :])
            nc.sync.dma_start(out=t0x[:, 2:KPG, :], in_=xt_d[0, :, 2:KPG, :])
            xt_gs[(0, 0)] = t0x
            nc.gpsimd.dma_start(out=wk_g[0][:, 0:2, :], in_=wk_d[:, 0:2, :])
            nc.gpsimd.dma_start(out=wk_g[0][:, 2:KPG, :], in_=wk_d[:, 2:KPG, :])
            nc.scalar.dma_start(out=wq_g[0][:, 0:2, :], in_=wq_d[:, 0:2, :])
            nc.scalar.dma_start(out=wq_g[0][:, 2:KPG, :], in_=wq_d[:, 2:KPG, :])
            for g in (1, 2):
                load_xt_group(0, g, eng=nc.sync)
            for g in range(1, KG):
                nc.gpsimd.dma_start(out=wk_g[g][:], in_=grp(wk_d, g))
                nc.scalar.dma_start(out=wq_g[g][:], in_=grp(wq_d, g))
            load_xt_group(0, 3, eng=nc.gpsimd)
            cc_sb = constp.tile([P, SEQ], BF16)
            ss_sb = constp.tile([P, SEQ], BF16)
            nc.gpsimd.dma_start(out=cc_sb[:], in_=cs_d[:])
            nc.gpsimd.dma_start(out=ss_sb[:], in_=sn_d[:])
            for g in range(KG):
                nc.gpsimd.dma_start(out=wv_g[g][:], in_=grp(wv_d, g))
            load_xt_group(1, 0, eng=nc.sync)
            load_xt_group(1, 2, eng=nc.sync)
            load_xt_group(1, 1, eng=nc.scalar)
            load_xt_group(1, 3, eng=nc.scalar)
            masks_sb = constp.tile([P, 4, W], BF16)
            nc.scalar.dma_start(out=masks_sb[:], in_=masks_d[:])
            ones_sb = constp.tile([P, P], BF16)
            nc.scalar.dma_start(out=ones_sb[:], in_=ones_d[:])
            wo_sb = wop.tile([P, KD, OUTC], BF16)

            ag_in = [dcc.tile([QH * HD, W], BF16, name=f"agin{w}")
                     for w in range(NW)]
            ag_out = [dcc.tile([DIM, W], BF16, addr_space="Shared",
                               name=f"agout{w}")
                      for w in range(NW)]

            def rope(ps, dst, s0):
                # ps rows 0:64 = even dims (x1), 64:128 = odd dims (x2);
                # cc/ss hold [cos;cos] and [sin;sin]. The psum is evicted
                # twice in bf16 — straight (psb, Activation engine) and
                # half-swapped (psw = [x2; x1], DVE; PSUM->SBUF copies may
                # shift the base partition) — so the rotation is two
                # full-width bf16 multiplies (2x DVE rate) plus the halved
                # sub/add, all on equal-base SBUF operands.
                cw = cc_sb[:, s0:s0 + W]
                sw = ss_sb[:, s0:s0 + W]
                psb = rtp.tile([P, W], BF16, tag="psb")
                psw = rtp.tile([P, W], BF16, tag="psw")
                nc.scalar.copy(psb[:], ps[:])
                nc.vector.tensor_copy(psw[0:64, :], ps[64:128, :])
                nc.vector.tensor_copy(psw[64:128, :], ps[0:64, :])
                t1 = rtp.tile([P, W], BF16, tag="t1")
                t2 = rtp.tile([P, W], BF16, tag="t2")
                nc.vector.tensor_mul(t1[:], psb[:], cw)      # [x1*c ; x2*c]
                nc.vector.tensor_mul(t2[:], psw[:], sw)      # [x2*s ; x1*s]
                nc.vector.tensor_sub(dst[0:64], t1[0:64, :], t2[0:64, :])
                nc.vector.tensor_add(dst[64:128], t1[64:128, :], t2[64:128, :])

            # steady-state psum pools: exactly 8 banks, with QKV/scores and
            # v/wo sharing slots (the streams interleave on the PE anyway).
            with tc.tile_pool(name="pmixs", bufs=2, space="PSUM") as pmixs, \
                 tc.tile_pool(name="pmixv", bufs=2, space="PSUM") as pmixv, \
                 tc.tile_pool(name="pso", bufs=2, space="PSUM") as pso, \
                 tc.tile_pool(name="pssum", bufs=2, space="PSUM") as pss, \
                 tc.tile_pool(name="ptp", bufs=4) as ptp, \
                 tc.tile_pool(name="sumt", bufs=3) as sumt, \
                 tc.tile_pool(name="attp", bufs=2) as attp:

                # zero the exp-tile buffers so the first diagonal mask
                # multiply never reads uninitialized SBUF (see docstring)
                for _ in range(4):
                    zt = ptp.tile([P, W], BF16, tag="pt")
                    nc.vector.memset(zt[:], 0.0)

                def emit_v(w):
                    for mm in range(W // P):
                        psv = pmixv.tile([P, OUTC], F32, tag="vw")
                        for k in range(KD):
                            g, kk = k // KPG, k % KPG
                            nc.tensor.matmul(
                                psv[:, 0:HD],
                                xt_gs[(w, g)][:, kk, mm * P:(mm + 1) * P],
                                wv_g[g][:, kk, :],
                                start=(k == 0), stop=(k == KD - 1))
                        nc.scalar.copy(v_sb[:, w * (W // P) + mm, :],
                                       psv[:, 0:HD])

                def emit_qkv_gouter(w):
                    # Window 0 runs contraction-major (k,q2,q3,q0,q1 per
                    # chunk) so PE consumption paces the initial DMA supply;
                    # later windows run chain-major (k chain first, then q
                    # heads in attention order) so each rope fires as soon as
                    # its chain completes. v last in both.
                    s0 = w * W
                    if w == 0:
                        pass  # window-1 groups already issued in the preamble
                    elif w + 1 < NW:
                        for g in range(KG):
                            load_xt_group(w + 1, g)
                    ps_k = pmixs.tile([P, W], F32, tag="s", name="kproj")
                    ps_q = {2: pso.tile([P, W], F32, tag="o", name="q2"),
                            3: pso.tile([P, W], F32, tag="o", name="q3"),
                            0: pss.tile([P, W], F32, tag="sum", name="q0"),
                            1: pss.tile([P, W], F32, tag="sum", name="q1")}

                    def kmm(g, kk):
                        k = g * KPG + kk
                        nc.tensor.matmul(ps_k[:], wk_g[g][:, kk, :],
                                         xt_gs[(w, g)][:, kk, :],
                                         start=(k == 0), stop=(k == KD - 1))

                    def qmm(m, g, kk):
                        k = g * KPG + kk
                        nc.tensor.matmul(ps_q[m][:],
                                         wq_g[g][:, kk, m * HD:(m + 1) * HD],
                                         xt_gs[(w, g)][:, kk, :],
                                         start=(k == 0), stop=(k == KD - 1))

                    if w == 0:
                        for g in range(KG):
                            for kk in range(KPG):
                                kmm(g, kk)
                                for m in HEAD_ORDER:
                                    qmm(m, g, kk)
                        rope(ps_k, krot[:, s0:s0 + W], s0)
                        for m in HEAD_ORDER:
                            rope(ps_q[m], qrot[:, m, s0:s0 + W], s0)
                    else:
                        for g in range(KG):
                            for kk in range(KPG):
                                kmm(g, kk)
                        rope(ps_k, krot[:, s0:s0 + W], s0)
                        for m in HEAD_ORDER:
                            for g in range(KG):
                                for kk in range(KPG):
                                    qmm(m, g, kk)
                            rope(ps_q[m], qrot[:, m, s0:s0 + W], s0)
                    emit_v(w)

                def emit_attention(qc):
                    q0 = qc * W
                    nkv = (W // P) * (qc + 1)
                    nquads = nkv // 4

                    for h in HEAD_ORDER:
                        ps_o = pso.tile([P, W], F32, tag="o")
                        ps_sum = pss.tile([P, W], F32, tag="sum")
                        for qd in range(nquads):
                            pts = []
                            pa = None
                            for ji in range(4):
                                jj = 4 * qd + ji
                                j0 = jj * P
                                cdiag = jj - (W // P) * qc
                                # live trapezoid of a diagonal block
                                lo = cdiag * P if cdiag > 0 else 0
                                ps_s = pmixs.tile([P, W], F32, tag="s")
                                nc.tensor.matmul(ps_s[:, lo:W],
                                                 krot[:, j0:j0 + P],
                                                 qrot[:, h, q0 + lo:q0 + W],
                                                 start=True, stop=True)
                                pt = ptp.tile([P, W], BF16, tag="pt")
                                nc.scalar.activation(pt[:, lo:W], ps_s[:, lo:W],
                                                     Exp, scale=SCALE)
                                if cdiag >= 0:
                                    # zeroes the masked triangle AND any stale
                                    # [0:lo) prefix left from buffer reuse
                                    nc.vector.tensor_mul(
                                        pt[:], pt[:], masks_sb[:, cdiag, :])
                                nc.tensor.matmul(ps_o[:, lo:W],
                                                 v_sb[:, jj, :], pt[:, lo:W],
                                                 start=(jj == 0),
                                                 stop=(jj == nkv - 1))
                                pts.append(pt)
                                if ji == 1:
                                    pa = sumt.tile([P, W], BF16, tag="pta")
                                    nc.vector.tensor_add(pa[:], pts[0][:],
                                                         pts[1][:])
                            pb = sumt.tile([P, W], BF16, tag="pta")
                            nc.vector.tensor_add(pb[:], pts[2][:], pts[3][:])
                            pq = sumt.tile([P, W], BF16, tag="pta")
                            nc.vector.tensor_add(pq[:], pa[:], pb[:])
                            nc.tensor.matmul(ps_sum[:], ones_sb[:], pq[:],
                                             start=(qd == 0),
                                             stop=(qd == nquads - 1))
                        # the ones stationary replicates the row sum across
                        # all 128 partitions — reciprocal full-width, no
                        # PE broadcast needed
                        inv_bc = attp.tile([P, W], F32, tag="invbc")
                        nc.vector.reciprocal_approx_fast(out=inv_bc[:],
                                                         in_=ps_sum[:])
                        at = attp.tile([P, W], BF16, tag="at", bufs=3)
                        nc.vector.tensor_mul(at[:], ps_o[:], inv_bc[:])
                        nc.sync.dma_start(out=ag_in[qc][h * HD:(h + 1) * HD, :],
                                          in_=at[:])
                    nc.gpsimd.collective_compute(
                        "AllGather", mybir.AluOpType.bypass,
                        replica_groups=[list(range(N_CORES))],
                        ins=[ag_in[qc][:]], outs=[ag_out[qc][:]])

                atfs: dict[int, bass.AP] = {}

                def load_atf(qc):
                    # single full-window DMA on the sync queue: its AG wait
                    # is dependency-chained behind the at-DMAs (same queue)
                    # that feed that AllGather, so the scheduler cannot
                    # head-of-line-block anything time-critical with it
                    atf = atfp.tile([P, KD, W], BF16, tag="atf")
                    nc.sync.dma_start(
                        out=atf[:],
                        in_=ag_out[qc][:].rearrange("(k p) n -> p k n", p=P))
                    atfs[qc] = atf

                def emit_wo(qc):
                    q0 = qc * W
                    atf = atfs.pop(qc)
                    for mm in range(W // P):
                        m0 = mm * P
                        ps = pmixv.tile([P, OUTC], F32, tag="vw")
                        for k in range(KD):
                            nc.tensor.matmul(ps[:],
                                             atf[:, k, m0:m0 + P],
                                             wo_sb[:, k, :],
                                             start=(k == 0), stop=(k == KD - 1))
                        ot = outp.tile([P, OUTC], F32, tag="ot")
                        nc.vector.tensor_copy(ot[:], ps[:])
                        nc.scalar.dma_start(out=out_d[q0 + m0:q0 + m0 + P, :],
                                            in_=ot[:])

                emit_qkv_gouter(0)
                emit_attention(0)
                emit_qkv_gouter(1)
                emit_attention(1)
                emit_qkv_gouter(2)
                emit_attention(2)
                nc.scalar.dma_start(out=wo_sb[:], in_=wo_d[:])
                emit_qkv_gouter(3)
                region_a.close()   # frees the weight/x^T pools for atf staging
                region_b = ExitStack()
                atfp = region_b.enter_context(tc.tile_pool(name="atfp", bufs=3, side="right"))
                outp = region_b.enter_context(tc.tile_pool(name="outp", bufs=3, side="right"))
                emit_attention(3)
                load_atf(0)
                load_atf(1)
                load_atf(2)
                load_atf(3)
                emit_wo(0)
                emit_wo(1)
                emit_wo(2)
                emit_wo(3)
                region_b.close()

    nc.compile()
    return nc


_NC_CACHE = None


def _get_nc():
    global _NC_CACHE
    if _NC_CACHE is None:
        _NC_CACHE = _build_nc()
    return _NC_CACHE


def make_in_maps(x, freqs_cos, freqs_sin, wq, wk, wv, wo):
    bf16 = ml_dtypes.bfloat16
    # half-split permutation: evens then odds within each head's 128 dims
    pidx = np.concatenate([np.arange(0, HD, 2), np.arange(1, HD, 2)])

    def ptile(w):
        # [DIM, cols] -> [P, KD, cols] so per-partition DMA runs are contiguous
        return np.ascontiguousarray(
            w.reshape(KD, P, w.shape[1]).transpose(1, 0, 2).astype(bf16))

    xt = np.ascontiguousarray(
        np.asarray(x).T.astype(bf16).reshape(KD, P, NW, W).transpose(2, 1, 0, 3))
    cs1 = np.asarray(freqs_cos).T.astype(np.float32)   # [64, SEQ]
    sn1 = np.asarray(freqs_sin).T.astype(np.float32)
    cs = np.ascontiguousarray(np.concatenate([cs1, cs1], axis=0).astype(bf16))
    sn = np.ascontiguousarray(np.concatenate([sn1, sn1], axis=0).astype(bf16))
    wq = np.asarray(wq)
    wk = np.asarray(wk)
    wv = np.asarray(wv)
    wo = np.asarray(wo)
    in_maps = []
    for core in range(N_CORES):
        q_cols = np.concatenate([h * HD + pidx
                                 for h in range(QH * core, QH * (core + 1))])
        in_maps.append({
            "xt": xt,
            "wq": ptile(wq[:, q_cols]),
            "wk": ptile(wk[:, core * HD + pidx]),
            "wv": ptile(wv[:, core * HD:(core + 1) * HD]),
            "wo": ptile(wo[:, core * OUTC:(core + 1) * OUTC]),
            "cs": cs,
            "sn": sn,
        })
    return in_maps


def kernel(x, freqs_cos, freqs_sin, wq, wk, wv, wo, _run_kwargs=None):
    in_maps = make_in_maps(x, freqs_cos, freqs_sin, wq, wk, wv, wo)
    nc = _get_nc()
    res = run_bass_kernel_spmd(nc, in_maps, list(range(N_CORES)),
                               **(_run_kwargs or {}))
    out = np.concatenate([res.results[i]["out"] for i in range(N_CORES)], axis=1)
    if _run_kwargs is not None:
        kernel.last_results = res
    return np.ascontiguousarray(out.astype(np.float32))
